# revision 45
# baseline (speedup 1.0000x reference)
"""Multi-head self-attention kernel for 8 Trainium2 NeuronCores.

Problem: B=2, S=2048, D=1024, H=16 heads, head_dim=64, fp32 in/out.
Sharding: core = (batch b, head-group g of 4 heads); b = core//4, g = core%4.
Each core computes its 4 heads' attention for its batch plus a partial
output projection (wo row-sharded); the host sums the 4 partials per batch
and adds the constant (bv @ wo + bo) row.

v2 dataflow (bf16 matmuls, ACT-exp is the floor at ~1.03us per k-tile):
  All projections (Q, K, V) run w-stationary / x-moving producing
  transposed [128, S] tiles; V's seq-major layout for PV is recovered with
  DMA xbar transposes (off the PE).  The whole kernel is one software
  pipeline: per k-tile step the PE emits [scores(kt), PV(kt-2)] plus ~2
  background matmuls popped from a global queue (K chunks, VT m1, K1/Q1,
  outproj pieces), so the exp stream on ScalarE never waits and the PE
  never idles.  Calls run pair-major: (0,0..3) then (1,0..3); outproj(qt)
  is hosted by call (1,qt+1), outproj(3) is the tail.
"""
from collections import deque

import numpy as np

import concourse.mybir as mybir
import concourse.tile as tile
from concourse import bacc
from concourse.bass_utils import run_bass_kernel_spmd

F32 = mybir.dt.float32
F32R = mybir.dt.float32r
BF16 = mybir.dt.bfloat16
EXP = mybir.ActivationFunctionType.Exp

S = 2048            # sequence length
D = 1024            # embed dim
HPC = 4             # heads per core
HD = 64             # head dim
GD = HPC * HD       # 256, per-core slice of D for QKV
NDK = D // 128      # 8 k-tiles over D
NKT = S // 128      # 16 k-tiles over S (attention contraction)
NQT = S // 512      # 4 q-tiles of 512

MMDT = BF16
N_BG = 2            # background matmuls per kt step

_CACHED = {}


def _np_mm():
    if MMDT == BF16:
        import ml_dtypes
        return ml_dtypes.bfloat16
    return np.float32


def build_nc():
    mmdt = MMDT
    nc = bacc.Bacc("TRN2", target_bir_lowering=False, debug=False, num_devices=8)
    xT = nc.dram_tensor("xT", [D, S], mmdt, kind="ExternalInput").ap()
    wq = nc.dram_tensor("wq", [128, NDK * GD], mmdt, kind="ExternalInput").ap()
    wk = nc.dram_tensor("wk", [128, NDK * GD], mmdt, kind="ExternalInput").ap()
    wv = nc.dram_tensor("wv", [128, NDK * GD], mmdt, kind="ExternalInput").ap()
    wo = nc.dram_tensor("wo", [GD, D], mmdt, kind="ExternalInput").ap()
    bq = nc.dram_tensor("bq", [128, 2], F32, kind="ExternalInput").ap()
    bk = nc.dram_tensor("bk", [128, 2], F32, kind="ExternalInput").ap()
    out = nc.dram_tensor("out", [S, D], F32, kind="ExternalOutput").ap()

    with tile.TileContext(nc) as tc:
        with tc.tile_pool(name="persist", bufs=1) as pw, \
             tc.tile_pool(name="projx", bufs=NDK) as pjx, \
             tc.tile_pool(name="projw", bufs=1) as pjw, \
             tc.tile_pool(name="scratch", bufs=2, space="PSUM") as scratch, \
             tc.tile_pool(name="stps", bufs=2, space="PSUM") as stps, \
             tc.tile_pool(name="ctps", bufs=1, space="PSUM") as ctps, \
             tc.tile_pool(name="ptp", bufs=4) as ptp, \
             tc.tile_pool(name="smalls", bufs=2) as smalls:

            # ---- long-lived tensors -------------------------------------
            qt_sb = [pw.tile([128, S], mmdt, tag=f"qt{m}", name=f"qt{m}")
                     for m in range(2)]
            kt_sb = [pw.tile([128, S], mmdt, tag=f"kt{m}", name=f"kt{m}")
                     for m in range(2)]
            # per-head VT [80, S]: rows 0-63 gd, row 64 ones, 65-79 pad
            # (xbar transpose needs partition count divisible by 16)
            vtt = [pw.tile([80, S], mmdt, tag=f"vtt{h}", name=f"vtt{h}")
                   for h in range(HPC)]
            # V' for PV: [seq-part, head, kt, hd+ones(+pad)]
            vp = pw.tile([128, HPC, NKT, 80], mmdt, tag="vp", name="vp")
            ctp_sb = [[pw.tile([128, 512], mmdt, tag=f"ctp{p}q{q}",
                               name=f"ctp{p}q{q}") for q in range(NQT)]
                      for p in range(2)]
            wop_sb = [pw.tile([128, D], mmdt, tag=f"wop{p}", name=f"wop{p}")
                      for p in range(2)]

            # m-major: [p, m(2), k(8), c(128)] so m-slices are contiguous
            wq_sb = pjw.tile([128, 2, NDK, 128], mmdt, tag="wq")
            wk_sb = pjw.tile([128, 2, NDK, 128], mmdt, tag="wk")
            wv_sb = pjw.tile([128, 2, NDK, 128], mmdt, tag="wv")
            bq_sb = pjw.tile([128, 2], F32, tag="bq")
            bk_sb = pjw.tile([128, 2], F32, tag="bk")

            # ---- input DMAs (m0 weight slices first, x split over both
            # hwdge queues, remaining weights after x) --------------------
            wq_v = wq.rearrange("p (m k c) -> p m k c", m=2, k=NDK)
            wk_v = wk.rearrange("p (m k c) -> p m k c", m=2, k=NDK)
            wv_v = wv.rearrange("p (m k c) -> p m k c", m=2, k=NDK)
            nc.sync.dma_start(wq_sb[:, 0], wq_v[:, 0])
            nc.sync.dma_start(wk_sb[:, 0], wk_v[:, 0])
            nc.sync.dma_start(wv_sb[:, 0], wv_v[:, 0])
            nc.scalar.dma_start(bq_sb[:], bq)
            nc.scalar.dma_start(bk_sb[:], bk)
            # all of x on the sync queue, back to back; deferred weights
            # trail it so nothing contends with x for HBM
            x_sb = [pjx.tile([128, S], mmdt, tag="xT", name=f"x{k}")
                    for k in range(NDK)]
            nc.sync.dma_start(x_sb[0][:, 0:512], xT[0:128, 0:512])
            nc.sync.dma_start(x_sb[0][:, 512:2048], xT[0:128, 512:2048])
            for k in range(1, NDK - 2):
                nc.sync.dma_start(x_sb[k][:], xT[128 * k:128 * k + 128, :])
            # last two k-tiles split by n-chunk so the wave's k7 matmuls
            # start on partial arrivals instead of the full-tile tail
            for k in (NDK - 2, NDK - 1):
                for n in range(4):
                    nc.sync.dma_start(
                        x_sb[k][:, 512 * n:512 * n + 512],
                        xT[128 * k:128 * k + 128, 512 * n:512 * n + 512])
            nc.sync.dma_start(wv_sb[:, 1], wv_v[:, 1])
            nc.sync.dma_start(wk_sb[:, 1], wk_v[:, 1])
            nc.sync.dma_start(wq_sb[:, 1], wq_v[:, 1])
            for p in range(2):
                nc.sync.dma_start(wop_sb[p][:], wo[128 * p:128 * p + 128, :])
            for h in range(HPC):
                nc.vector.memset(vtt[h][64:80, :], 1.0)

            # ---- background work queue ----------------------------------
            bg = deque()

            def bg_pop(n):
                for _ in range(n):
                    if bg:
                        bg.popleft()()

            def emit_transposes(m, n):
                # vtt heads 2m, 2m+1 chunk n -> vp s-tiles 4n..4n+4
                for h in range(2):
                    head = 2 * m + h
                    nc.sync.dma_start_transpose(
                        vp[:, head, 4 * n:4 * n + 4, :],
                        vtt[head][0:80, 512 * n:512 * n + 512])

            def push_chain(kind, m, n):
                """8 accumulating matmuls + evacuation, as bg closures.
                kind: 'q'/'k' (bias-add into qt/kt m-tile) or 'v' (copy
                into vtt + transposes)."""
                w_sb = {"q": wq_sb, "k": wk_sb, "v": wv_sb}[kind]
                holder = {}

                def mk(k):
                    def f():
                        if k == 0:
                            holder["ps"] = scratch.tile(
                                [128, 512], F32, tag="sc", name="bgps")
                        nc.tensor.matmul(
                            holder["ps"][:], w_sb[:, m, k, :],
                            x_sb[k][:, 512 * n:512 * n + 512],
                            start=(k == 0), stop=(k == NDK - 1))
                        if k == NDK - 1:
                            ps = holder["ps"]
                            if kind == "q":
                                nc.vector.tensor_scalar_add(
                                    qt_sb[m][:, 512 * n:512 * n + 512],
                                    ps[:], bq_sb[:, m:m + 1])
                            elif kind == "k":
                                nc.vector.tensor_scalar_add(
                                    kt_sb[m][:, 512 * n:512 * n + 512],
                                    ps[:], bk_sb[:, m:m + 1])
                            else:
                                for h in range(2):
                                    nc.vector.tensor_copy(
                                        vtt[2 * m + h][0:64,
                                                       512 * n:512 * n + 512],
                                        ps[64 * h:64 * h + 64, :])
                                emit_transposes(m, n)
                    return f
                for k in range(NDK):
                    bg.append(mk(k))

            def push_outproj(qt):
                """8 pieces of 2 matmuls; osb evac + DMA per si."""
                holder = {}

                def mk(si_l, n, p):
                    si = 4 * qt + si_l

                    def f():
                        if p == 0:
                            holder[(si_l, n)] = scratch.tile(
                                [128, 512], F32, tag="sc", name="ops")
                        op = holder[(si_l, n)]
                        nc.tensor.matmul(
                            op[:], ctp_sb[p][qt][:, 128 * si_l:128 * si_l + 128],
                            wop_sb[p][:, 512 * n:512 * n + 512],
                            start=(p == 0), stop=(p == 1))
                        if p == 1:
                            if n == 0:
                                holder["osb", si_l] = smalls.tile(
                                    [128, 1024], F32, tag="osb", name="osb")
                            osb = holder["osb", si_l]
                            nc.vector.tensor_copy(
                                osb[:, 512 * n:512 * n + 512], op[:])
                            if n == 1:
                                nc.sync.dma_start(
                                    out[128 * si:128 * si + 128, :], osb[:])
                    return f
                for si_l in range(4):
                    for n in range(2):
                        for p in range(2):
                            bg.append(mk(si_l, n, p))

            # ---- attention ----------------------------------------------
            def normalize(pair, qt, ct_ps, tail=False):
                for par in (1, 0):
                    # denominator row straight from PSUM so the reciprocal
                    # chain overlaps the big ct copy
                    rden = smalls.tile([1, 512], F32, tag="rden", name="rden")
                    nc.vector.tensor_copy(rden[:], ct_ps[par][64:65, :])
                    rrec = smalls.tile([1, 512], F32, tag="rrec", name="rrec")
                    nc.vector.reciprocal_approx_fast(rrec[:], rden[:])
                    rb = smalls.tile([64, 512], F32, tag="rb", name="rb")
                    nc.gpsimd.partition_broadcast(rb[:], rrec[:])
                    ctsb = smalls.tile([65, 512], F32, tag="ctsb",
                                       name="ctsb")
                    if tail and par == 0:
                        nc.scalar.copy(ctsb[0:64, :], ct_ps[par][0:64, :])
                    else:
                        nc.vector.tensor_copy(ctsb[0:64, :],
                                              ct_ps[par][0:64, :])
                    if par == 0:
                        nc.vector.tensor_mul(
                            ctp_sb[pair][qt][0:64, :], ctsb[0:64, :], rb[:])
                    else:
                        todd = smalls.tile([64, 512], mmdt, tag="todd",
                                           name="todd")
                        nc.vector.tensor_mul(todd[:], ctsb[0:64, :], rb[:])
                        nc.sync.dma_start(
                            ctp_sb[pair][qt][64:128, :], todd[:])

            # ---- global step stream -------------------------------------
            # One flat stream of (call, kt) steps; each step emits its
            # scores+exp, then the PV of the step two back, then bg work.
            # Each call's kt=0 step is swapped one slot earlier (before the
            # previous call's kt=15) so the exp streams of adjacent calls
            # interleave at the boundary and every gate is satisfied early.
            calls = [(0, 0), (0, 1), (0, 2), (0, 3),
                     (1, 0), (1, 1), (1, 2), (1, 3)]
            ct_of = {}
            pts = {}

            def emit_S(ci, kt):
                pair, qt = calls[ci]
                q0 = 512 * qt
                st = stps.tile([128, 1024], F32, tag="st", name="st")
                for par in range(2):
                    p0 = 64 * par
                    nc.tensor.matmul(
                        st[:, 512 * par:512 * par + 512],
                        kt_sb[pair][p0:p0 + 64, 128 * kt:128 * kt + 128],
                        qt_sb[pair][p0:p0 + 64, q0:q0 + 512],
                        start=True, stop=True, tile_position=(p0, 0))
                pt = ptp.tile([128, 1024], mmdt, tag="pt", name="pt")
                nc.scalar.activation(pt[:], st[:], EXP, scale=0.125)
                pts[(ci, kt)] = pt

            def emit_PV(ci, kt):
                pair, qt = calls[ci]
                if ci not in ct_of:
                    ct_of[ci] = [ctps.tile([65, 512], F32, tag=f"ctp{par}",
                                           name=f"ctps{par}")
                                 for par in range(2)]
                ct_ps = ct_of[ci]
                pt = pts.pop((ci, kt))
                for par in range(2):
                    nc.tensor.matmul(
                        ct_ps[par][:, :],
                        vp[:, 2 * pair + par, kt, 0:HD + 1],
                        pt[:, 512 * par:512 * par + 512],
                        start=(kt == 0), stop=(kt == NKT - 1))
                if kt == NKT - 1:
                    pair_, qt_ = calls[ci]
                    normalize(pair_, qt_, ct_ps, tail=(ci == len(calls) - 1))

            def run_stream(pushes):
                pv_list = []
                for ci in range(len(calls)):
                    for kt in range(NKT):
                        pv_list.append((ci, kt))
                steps = list(pv_list)
                for g, (ci, kt) in enumerate(steps):
                    if (ci, kt) in pushes:
                        for thunk in pushes[(ci, kt)]:
                            thunk()
                    emit_S(ci, kt)
                    if g >= 2:
                        emit_PV(*pv_list[g - 2])
                    bg_pop(N_BG + 2 if g < 2 else N_BG)
                return pv_list[-2:]

            # ---- startup: x-paced wave -----------------------------------
            # 6 concurrent chains: Qm0n0, Km0n0 (scratch ring), Km0 n1/n2
            # (st slot A) + VTm0 n0/n1 (st slot B).  Evacuations split
            # across DVE and ACT so S(0)/S(1) unblock fast: slot A frees
            # via two ACT bias-adds, slot B via one copy on each engine.
            st_w1 = stps.tile([128, 1024], F32, tag="st", name="stw1")
            st_w2 = stps.tile([128, 1024], F32, tag="st", name="stw2")
            ps_q0 = scratch.tile([128, 512], F32, tag="sc", name="psq0")
            ps_k0 = scratch.tile([128, 512], F32, tag="sc", name="psk0")
            wave = [
                ("q", 0, 0, ps_q0[:]),
                ("k", 0, 0, ps_k0[:]),
                ("k", 0, 1, st_w1[:, 0:512]),
                ("k", 0, 2, st_w1[:, 512:1024]),
                ("v", 0, 0, st_w2[:, 0:512]),
                ("v", 0, 1, st_w2[:, 512:1024]),
            ]
            wsel = {"q": wq_sb, "k": wk_sb, "v": wv_sb}
            for k in range(NDK):
                for kind, m, n, ps in wave:
                    nc.tensor.matmul(
                        ps, wsel[kind][:, m, k, :],
                        x_sb[k][:, 512 * n:512 * n + 512],
                        start=(k == 0), stop=(k == NDK - 1))
            IDENT = mybir.ActivationFunctionType.Identity
            nc.vector.tensor_scalar_add(
                qt_sb[0][:, 0:512], ps_q0[:], bq_sb[:, 0:1])
            nc.vector.tensor_scalar_add(
                kt_sb[0][:, 0:512], ps_k0[:], bk_sb[:, 0:1])
            for n in (1, 2):
                nc.scalar.activation(
                    kt_sb[0][:, 512 * n:512 * n + 512],
                    st_w1[:, 512 * (n - 1):512 * (n - 1) + 512],
                    IDENT, bias=bk_sb[:, 0:1])
            for n in (0, 1):
                ps = st_w2[:, 512 * n:512 * n + 512]
                nc.vector.tensor_copy(vtt[0][0:64, 512 * n:512 * n + 512],
                                      ps[0:64, :])
                nc.scalar.copy(vtt[1][0:64, 512 * n:512 * n + 512],
                               ps[64:128, :])
                emit_transposes(0, n)

            # ---- emission schedule --------------------------------------
            pushes = {
                (0, 0): [lambda: push_chain("k", 0, 3),
                         lambda: push_chain("v", 0, 2),
                         lambda: push_chain("v", 0, 3),
                         lambda: push_chain("q", 0, 1)],
                (1, 0): [lambda: push_chain("q", 0, 2),
                         lambda: push_chain("q", 0, 3)] +
                        [lambda n=n: push_chain("v", 1, n) for n in range(4)],
                (2, 0): [lambda n=n: push_chain("k", 1, n) for n in range(4)],
                (3, 0): [lambda n=n: push_chain("q", 1, n) for n in range(4)],
                # outproj(qt) pushed at kt=1 of call 5+qt so its pops come
                # after the producing normalize is emitted (same-step PV)
                (5, 1): [lambda: push_outproj(0)],
                (6, 1): [lambda: push_outproj(1)],
                (7, 1): [lambda: push_outproj(2)],
            }
            last2 = run_stream(pushes)
            while bg:
                bg.popleft()()

            # ---- dense tail outproj(3): st ring is free post-attention;
            # 6 concurrent PSUM slots, p0 halves prefetched before the
            # last call's normalize, ACT+DVE evacuation.
            qt = NQT - 1
            emit_PV(*last2[0])
            stt = [stps.tile([128, 1024], F32, tag="st", name="stt")
                   for _ in range(2)]
            slots = ([scratch.tile([128, 512], F32, tag="sc", name="tsl")
                      for _ in range(2)] +
                     [stt[i][:, 512 * j:512 * j + 512]
                      for i in range(2) for j in range(2)])
            pieces = [(si_l, n) for si_l in range(4) for n in range(2)]
            for pi in range(6):
                si_l, n = pieces[pi]
                nc.tensor.matmul(
                    slots[pi], ctp_sb[0][qt][:, 128 * si_l:128 * si_l + 128],
                    wop_sb[0][:, 512 * n:512 * n + 512],
                    start=True, stop=False)
            emit_PV(*last2[1])
            osbs = [smalls.tile([128, 1024], F32, tag="osbt", bufs=4,
                                name="osbt") for _ in range(4)]
            for pi, (si_l, n) in enumerate(pieces):
                op = slots[pi % 6]
                si = 4 * qt + si_l
                if pi >= 6:
                    nc.tensor.matmul(
                        op, ctp_sb[0][qt][:, 128 * si_l:128 * si_l + 128],
                        wop_sb[0][:, 512 * n:512 * n + 512],
                        start=True, stop=False)
                nc.tensor.matmul(
                    op, ctp_sb[1][qt][:, 128 * si_l:128 * si_l + 128],
                    wop_sb[1][:, 512 * n:512 * n + 512],
                    start=False, stop=True)
                eng = nc.scalar.copy if (pi % 2 == 0) else \
                    (lambda d, s: nc.vector.tensor_copy(d, s))
                eng(osbs[si_l][:, 512 * n:512 * n + 512], op)
                if n == 1:
                    nc.sync.dma_start(out[128 * si:128 * si + 128, :],
                                      osbs[si_l][:])

    nc.compile()
    return nc


def _get_nc():
    if MMDT not in _CACHED:
        _CACHED[MMDT] = build_nc()
    return _CACHED[MMDT]


def shard_inputs(x, wq, bq, wk, bk, wv, bv, wo, bo):
    np_mm = _np_mm()
    x = np.asarray(x, dtype=np.float32)
    wq, bq = np.asarray(wq, np.float32), np.asarray(bq, np.float32)
    wk, bk = np.asarray(wk, np.float32), np.asarray(bk, np.float32)
    wv = np.asarray(wv, np.float32)
    wo = np.asarray(wo, np.float32)

    def wlayout(w):  # [D, GD] -> [128, NDK*GD] with (p, m, k, c) order
        return np.ascontiguousarray(
            w.reshape(NDK, 128, 2, 128).transpose(1, 2, 0, 3)
            .reshape(128, NDK * GD)
        ).astype(np_mm)

    def blayout(b):  # [GD] -> [128, 2] with (p, m) order
        return np.ascontiguousarray(b.reshape(2, 128).T).astype(np.float32)

    in_maps = []
    for core in range(8):
        b, g = core // 4, core % 4
        hs = slice(g * GD, (g + 1) * GD)
        in_maps.append({
            "xT": np.ascontiguousarray(x[b].T).astype(np_mm),
            "wq": wlayout(wq[:, hs]),
            "wk": wlayout(wk[:, hs]),
            "wv": wlayout(wv[:, hs]),
            "wo": np.ascontiguousarray(wo[hs, :]).astype(np_mm),
            "bq": blayout(bq[hs]),
            "bk": blayout(bk[hs]),
        })
    return in_maps


def kernel(x, wq, bq, wk, bk, wv, bv, wo, bo, _trace=False, _tracekw=None):
    nc = _get_nc()
    in_maps = shard_inputs(x, wq, bq, wk, bk, wv, bv, wo, bo)
    kw = dict(_tracekw or {})
    res = run_bass_kernel_spmd(nc, in_maps, core_ids=list(range(8)),
                               trace=_trace, **kw)
    kernel._last_result = res
    # constant row: bv @ wo + bo (exact in float64, folded on host)
    bv64 = np.asarray(bv, np.float64)
    wo64 = np.asarray(wo, np.float64)
    bo64 = np.asarray(bo, np.float64)
    const_row = bv64 @ wo64 + bo64
    full = np.empty((2, S, D), np.float32)
    for b in range(2):
        acc = np.zeros((S, D), np.float64)
        for g in range(4):
            acc += res.results[4 * b + g]["out"]
        full[b] = (acc + const_row).astype(np.float32)
    return full


kernel._last_result = None


# revision 46
# speedup vs baseline: 1.0165x; 1.0165x over previous
"""Multi-head self-attention kernel for 8 Trainium2 NeuronCores.

Problem: B=2, S=2048, D=1024, H=16 heads, head_dim=64, fp32 in/out.
Sharding: core = (batch b, head-group g of 4 heads); b = core//4, g = core%4.
Each core computes its 4 heads' attention for its batch plus a partial
output projection (wo row-sharded); the host sums the 4 partials per batch
and adds the constant (bv @ wo + bo) row.

v2 dataflow (bf16 matmuls, ACT-exp is the floor at ~1.03us per k-tile):
  All projections (Q, K, V) run w-stationary / x-moving producing
  transposed [128, S] tiles; V's seq-major layout for PV is recovered with
  DMA xbar transposes (off the PE).  The whole kernel is one software
  pipeline: per k-tile step the PE emits [scores(kt), PV(kt-2)] plus ~2
  background matmuls popped from a global queue (K chunks, VT m1, K1/Q1,
  outproj pieces), so the exp stream on ScalarE never waits and the PE
  never idles.  Calls run pair-major: (0,0..3) then (1,0..3); outproj(qt)
  is hosted by call (1,qt+1), outproj(3) is the tail.
"""
from collections import deque

import numpy as np

import concourse.mybir as mybir
import concourse.tile as tile
from concourse import bacc
from concourse.bass_utils import run_bass_kernel_spmd

F32 = mybir.dt.float32
F32R = mybir.dt.float32r
BF16 = mybir.dt.bfloat16
EXP = mybir.ActivationFunctionType.Exp

S = 2048            # sequence length
D = 1024            # embed dim
HPC = 4             # heads per core
HD = 64             # head dim
GD = HPC * HD       # 256, per-core slice of D for QKV
NDK = D // 128      # 8 k-tiles over D
NKT = S // 128      # 16 k-tiles over S (attention contraction)
NQT = S // 512      # 4 q-tiles of 512

MMDT = BF16
N_BG = 2            # background matmuls per kt step

_CACHED = {}


def _np_mm():
    if MMDT == BF16:
        import ml_dtypes
        return ml_dtypes.bfloat16
    return np.float32


def build_nc():
    mmdt = MMDT
    nc = bacc.Bacc("TRN2", target_bir_lowering=False, debug=False, num_devices=8)
    xT = nc.dram_tensor("xT", [D, S], mmdt, kind="ExternalInput").ap()
    wq = nc.dram_tensor("wq", [128, NDK * GD], mmdt, kind="ExternalInput").ap()
    wk = nc.dram_tensor("wk", [128, NDK * GD], mmdt, kind="ExternalInput").ap()
    wv = nc.dram_tensor("wv", [128, NDK * GD], mmdt, kind="ExternalInput").ap()
    wo = nc.dram_tensor("wo", [GD, D], mmdt, kind="ExternalInput").ap()
    bq = nc.dram_tensor("bq", [128, 2], F32, kind="ExternalInput").ap()
    bk = nc.dram_tensor("bk", [128, 2], F32, kind="ExternalInput").ap()
    out = nc.dram_tensor("out", [S, D], F32, kind="ExternalOutput").ap()

    with tile.TileContext(nc) as tc:
        with tc.tile_pool(name="persist", bufs=1) as pw, \
             tc.tile_pool(name="projx", bufs=NDK) as pjx, \
             tc.tile_pool(name="projw", bufs=1) as pjw, \
             tc.tile_pool(name="scratch", bufs=2, space="PSUM") as scratch, \
             tc.tile_pool(name="stps", bufs=2, space="PSUM") as stps, \
             tc.tile_pool(name="ctps", bufs=1, space="PSUM") as ctps, \
             tc.tile_pool(name="ptp", bufs=4) as ptp, \
             tc.tile_pool(name="smalls", bufs=2) as smalls:

            # ---- long-lived tensors -------------------------------------
            qt_sb = [pw.tile([128, S], mmdt, tag=f"qt{m}", name=f"qt{m}")
                     for m in range(2)]
            kt_sb = [pw.tile([128, S], mmdt, tag=f"kt{m}", name=f"kt{m}")
                     for m in range(2)]
            # per-head VT [80, S]: rows 0-63 gd, row 64 ones, 65-79 pad
            # (xbar transpose needs partition count divisible by 16)
            vtt = [pw.tile([80, S], mmdt, tag=f"vtt{h}", name=f"vtt{h}")
                   for h in range(HPC)]
            # V' for PV: [seq-part, head, kt, hd+ones(+pad)]
            vp = pw.tile([128, HPC, NKT, 80], mmdt, tag="vp", name="vp")
            ctp_sb = [[pw.tile([128, 512], mmdt, tag=f"ctp{p}q{q}",
                               name=f"ctp{p}q{q}") for q in range(NQT)]
                      for p in range(2)]
            wop_sb = [pw.tile([128, D], mmdt, tag=f"wop{p}", name=f"wop{p}")
                      for p in range(2)]

            # m-major: [p, m(2), k(8), c(128)] so m-slices are contiguous
            wq_sb = pjw.tile([128, 2, NDK, 128], mmdt, tag="wq")
            wk_sb = pjw.tile([128, 2, NDK, 128], mmdt, tag="wk")
            wv_sb = pjw.tile([128, 2, NDK, 128], mmdt, tag="wv")
            bq_sb = pjw.tile([128, 2], F32, tag="bq")
            bk_sb = pjw.tile([128, 2], F32, tag="bk")

            # ---- input DMAs (m0 weight slices first, x split over both
            # hwdge queues, remaining weights after x) --------------------
            wq_v = wq.rearrange("p (m k c) -> p m k c", m=2, k=NDK)
            wk_v = wk.rearrange("p (m k c) -> p m k c", m=2, k=NDK)
            wv_v = wv.rearrange("p (m k c) -> p m k c", m=2, k=NDK)
            nc.sync.dma_start(wq_sb[:, 0], wq_v[:, 0])
            nc.sync.dma_start(wk_sb[:, 0], wk_v[:, 0])
            nc.sync.dma_start(wv_sb[:, 0], wv_v[:, 0])
            nc.scalar.dma_start(bq_sb[:], bq)
            nc.scalar.dma_start(bk_sb[:], bk)
            # all of x on the sync queue, back to back; deferred weights
            # trail it so nothing contends with x for HBM
            x_sb = [pjx.tile([128, S], mmdt, tag="xT", name=f"x{k}")
                    for k in range(NDK)]
            nc.sync.dma_start(x_sb[0][:, 0:512], xT[0:128, 0:512])
            nc.sync.dma_start(x_sb[0][:, 512:2048], xT[0:128, 512:2048])
            for k in range(1, NDK - 2):
                nc.sync.dma_start(x_sb[k][:], xT[128 * k:128 * k + 128, :])
            # last two k-tiles split by n-chunk so the wave's k7 matmuls
            # start on partial arrivals instead of the full-tile tail
            for k in (NDK - 2, NDK - 1):
                for n in range(4):
                    nc.sync.dma_start(
                        x_sb[k][:, 512 * n:512 * n + 512],
                        xT[128 * k:128 * k + 128, 512 * n:512 * n + 512])
            nc.sync.dma_start(wv_sb[:, 1], wv_v[:, 1])
            nc.sync.dma_start(wk_sb[:, 1], wk_v[:, 1])
            nc.sync.dma_start(wq_sb[:, 1], wq_v[:, 1])
            for p in range(2):
                nc.sync.dma_start(wop_sb[p][:], wo[128 * p:128 * p + 128, :])
            for h in range(HPC):
                nc.vector.memset(vtt[h][64:80, :], 1.0)

            # ---- background work queue ----------------------------------
            bg = deque()

            def bg_pop(n):
                for _ in range(n):
                    if bg:
                        bg.popleft()()

            def emit_transposes(m, n):
                # vtt heads 2m, 2m+1 chunk n -> vp s-tiles 4n..4n+4
                for h in range(2):
                    head = 2 * m + h
                    nc.sync.dma_start_transpose(
                        vp[:, head, 4 * n:4 * n + 4, :],
                        vtt[head][0:80, 512 * n:512 * n + 512])

            def push_chain(kind, m, n):
                """8 accumulating matmuls + evacuation, as bg closures.
                kind: 'q'/'k' (bias-add into qt/kt m-tile) or 'v' (copy
                into vtt + transposes)."""
                w_sb = {"q": wq_sb, "k": wk_sb, "v": wv_sb}[kind]
                holder = {}

                def mk(k):
                    def f():
                        if k == 0:
                            holder["ps"] = scratch.tile(
                                [128, 512], F32, tag="sc", name="bgps")
                        nc.tensor.matmul(
                            holder["ps"][:], w_sb[:, m, k, :],
                            x_sb[k][:, 512 * n:512 * n + 512],
                            start=(k == 0), stop=(k == NDK - 1))
                        if k == NDK - 1:
                            ps = holder["ps"]
                            if kind == "q":
                                nc.vector.tensor_scalar_add(
                                    qt_sb[m][:, 512 * n:512 * n + 512],
                                    ps[:], bq_sb[:, m:m + 1])
                            elif kind == "k":
                                nc.vector.tensor_scalar_add(
                                    kt_sb[m][:, 512 * n:512 * n + 512],
                                    ps[:], bk_sb[:, m:m + 1])
                            else:
                                for h in range(2):
                                    nc.vector.tensor_copy(
                                        vtt[2 * m + h][0:64,
                                                       512 * n:512 * n + 512],
                                        ps[64 * h:64 * h + 64, :])
                                emit_transposes(m, n)
                    return f
                for k in range(NDK):
                    bg.append(mk(k))

            def push_outproj(qt):
                """8 pieces of 2 matmuls; osb evac + DMA per si."""
                holder = {}

                def mk(si_l, n, p):
                    si = 4 * qt + si_l

                    def f():
                        if p == 0:
                            holder[(si_l, n)] = scratch.tile(
                                [128, 512], F32, tag="sc", name="ops")
                        op = holder[(si_l, n)]
                        nc.tensor.matmul(
                            op[:], ctp_sb[p][qt][:, 128 * si_l:128 * si_l + 128],
                            wop_sb[p][:, 512 * n:512 * n + 512],
                            start=(p == 0), stop=(p == 1))
                        if p == 1:
                            if n == 0:
                                holder["osb", si_l] = smalls.tile(
                                    [128, 1024], F32, tag="osb", name="osb")
                            osb = holder["osb", si_l]
                            nc.vector.tensor_copy(
                                osb[:, 512 * n:512 * n + 512], op[:])
                            if n == 1:
                                nc.sync.dma_start(
                                    out[128 * si:128 * si + 128, :], osb[:])
                    return f
                for si_l in range(4):
                    for n in range(2):
                        for p in range(2):
                            bg.append(mk(si_l, n, p))

            # ---- attention ----------------------------------------------
            def normalize(pair, qt, ct_ps, tail=False):
                for par in (1, 0):
                    # denominator row straight from PSUM so the reciprocal
                    # chain overlaps the big ct copy
                    rden = smalls.tile([1, 512], F32, tag="rden", name="rden")
                    nc.vector.tensor_copy(rden[:], ct_ps[par][64:65, :])
                    rrec = smalls.tile([1, 512], F32, tag="rrec", name="rrec")
                    nc.vector.reciprocal_approx_fast(rrec[:], rden[:])
                    rb = smalls.tile([64, 512], F32, tag="rb", name="rb")
                    nc.gpsimd.partition_broadcast(rb[:], rrec[:])
                    ctsb = smalls.tile([65, 512], F32, tag="ctsb",
                                       name="ctsb")
                    if tail and par == 0:
                        nc.scalar.copy(ctsb[0:64, :], ct_ps[par][0:64, :])
                    else:
                        nc.vector.tensor_copy(ctsb[0:64, :],
                                              ct_ps[par][0:64, :])
                    if par == 0:
                        nc.vector.tensor_mul(
                            ctp_sb[pair][qt][0:64, :], ctsb[0:64, :], rb[:])
                    else:
                        todd = smalls.tile([64, 512], mmdt, tag="todd",
                                           name="todd")
                        nc.vector.tensor_mul(todd[:], ctsb[0:64, :], rb[:])
                        nc.sync.dma_start(
                            ctp_sb[pair][qt][64:128, :], todd[:])

            # ---- global step stream -------------------------------------
            # One flat stream of (call, kt) steps; each step emits its
            # scores+exp, then the PV of the step two back, then bg work.
            # Each call's kt=0 step is swapped one slot earlier (before the
            # previous call's kt=15) so the exp streams of adjacent calls
            # interleave at the boundary and every gate is satisfied early.
            calls = [(0, 0), (0, 1), (0, 2), (0, 3),
                     (1, 0), (1, 1), (1, 2), (1, 3)]
            ct_of = {}
            pts = {}

            def emit_S(ci, kt):
                pair, qt = calls[ci]
                q0 = 512 * qt
                st = stps.tile([128, 1024], F32, tag="st", name="st")
                for par in range(2):
                    p0 = 64 * par
                    nc.tensor.matmul(
                        st[:, 512 * par:512 * par + 512],
                        kt_sb[pair][p0:p0 + 64, 128 * kt:128 * kt + 128],
                        qt_sb[pair][p0:p0 + 64, q0:q0 + 512],
                        start=True, stop=True, tile_position=(p0, 0))
                pt = ptp.tile([128, 1024], mmdt, tag="pt", name="pt")
                nc.scalar.activation(pt[:], st[:], EXP, scale=0.125)
                pts[(ci, kt)] = pt

            def emit_PV(ci, kt):
                pair, qt = calls[ci]
                if ci not in ct_of:
                    ct_of[ci] = [ctps.tile([65, 512], F32, tag=f"ctp{par}",
                                           name=f"ctps{par}")
                                 for par in range(2)]
                ct_ps = ct_of[ci]
                pt = pts.pop((ci, kt))
                for par in range(2):
                    nc.tensor.matmul(
                        ct_ps[par][:, :],
                        vp[:, 2 * pair + par, kt, 0:HD + 1],
                        pt[:, 512 * par:512 * par + 512],
                        start=(kt == 0), stop=(kt == NKT - 1))
                if kt == NKT - 1:
                    pair_, qt_ = calls[ci]
                    normalize(pair_, qt_, ct_ps, tail=(ci == len(calls) - 1))

            def run_stream(pushes):
                pv_list = []
                for ci in range(len(calls)):
                    for kt in range(NKT):
                        pv_list.append((ci, kt))
                steps = list(pv_list)
                for g, (ci, kt) in enumerate(steps):
                    if (ci, kt) in pushes:
                        for thunk in pushes[(ci, kt)]:
                            thunk()
                    emit_S(ci, kt)
                    if g >= 2:
                        emit_PV(*pv_list[g - 2])
                    # keep the call-boundary steps free of background work
                    # so the next call's score pairs dispatch immediately
                    bg_pop(0 if kt in (0, 1) else
                           (2 * N_BG if kt in (2, 3) else N_BG))
                return pv_list[-2:]

            # ---- startup: x-paced wave -----------------------------------
            # 6 concurrent chains: Qm0n0, Km0n0 (scratch ring), Km0 n1/n2
            # (st slot A) + VTm0 n0/n1 (st slot B).  Evacuations split
            # across DVE and ACT so S(0)/S(1) unblock fast: slot A frees
            # via two ACT bias-adds, slot B via one copy on each engine.
            st_w1 = stps.tile([128, 1024], F32, tag="st", name="stw1")
            st_w2 = stps.tile([128, 1024], F32, tag="st", name="stw2")
            ps_q0 = scratch.tile([128, 512], F32, tag="sc", name="psq0")
            ps_k0 = scratch.tile([128, 512], F32, tag="sc", name="psk0")
            wave = [
                ("q", 0, 0, ps_q0[:]),
                ("k", 0, 0, ps_k0[:]),
                ("k", 0, 1, st_w1[:, 0:512]),
                ("k", 0, 2, st_w1[:, 512:1024]),
                ("v", 0, 0, st_w2[:, 0:512]),
                ("v", 0, 1, st_w2[:, 512:1024]),
            ]
            wsel = {"q": wq_sb, "k": wk_sb, "v": wv_sb}
            for k in range(NDK):
                for kind, m, n, ps in wave:
                    nc.tensor.matmul(
                        ps, wsel[kind][:, m, k, :],
                        x_sb[k][:, 512 * n:512 * n + 512],
                        start=(k == 0), stop=(k == NDK - 1))
            IDENT = mybir.ActivationFunctionType.Identity
            nc.vector.tensor_scalar_add(
                qt_sb[0][:, 0:512], ps_q0[:], bq_sb[:, 0:1])
            nc.vector.tensor_scalar_add(
                kt_sb[0][:, 0:512], ps_k0[:], bk_sb[:, 0:1])
            for n in (1, 2):
                nc.scalar.activation(
                    kt_sb[0][:, 512 * n:512 * n + 512],
                    st_w1[:, 512 * (n - 1):512 * (n - 1) + 512],
                    IDENT, bias=bk_sb[:, 0:1])
            for n in (0, 1):
                ps = st_w2[:, 512 * n:512 * n + 512]
                nc.vector.tensor_copy(vtt[0][0:64, 512 * n:512 * n + 512],
                                      ps[0:64, :])
                nc.scalar.copy(vtt[1][0:64, 512 * n:512 * n + 512],
                               ps[64:128, :])
                emit_transposes(0, n)

            # ---- emission schedule --------------------------------------
            pushes = {
                (0, 0): [lambda: push_chain("k", 0, 3),
                         lambda: push_chain("v", 0, 2),
                         lambda: push_chain("v", 0, 3),
                         lambda: push_chain("q", 0, 1)],
                (1, 0): [lambda: push_chain("q", 0, 2),
                         lambda: push_chain("q", 0, 3)] +
                        [lambda n=n: push_chain("v", 1, n) for n in range(4)],
                (2, 0): [lambda n=n: push_chain("k", 1, n) for n in range(4)],
                (3, 0): [lambda n=n: push_chain("q", 1, n) for n in range(4)],
                # outproj(qt) pushed at kt=1 of call 5+qt so its pops come
                # after the producing normalize is emitted (same-step PV)
                (5, 1): [lambda: push_outproj(0)],
                (6, 1): [lambda: push_outproj(1)],
                (7, 1): [lambda: push_outproj(2)],
            }
            last2 = run_stream(pushes)
            while bg:
                bg.popleft()()

            # ---- dense tail outproj(3): st ring is free post-attention;
            # 6 concurrent PSUM slots, p0 halves prefetched before the
            # last call's normalize, ACT+DVE evacuation.
            qt = NQT - 1
            emit_PV(*last2[0])
            stt = [stps.tile([128, 1024], F32, tag="st", name="stt")
                   for _ in range(2)]
            slots = ([scratch.tile([128, 512], F32, tag="sc", name="tsl")
                      for _ in range(2)] +
                     [stt[i][:, 512 * j:512 * j + 512]
                      for i in range(2) for j in range(2)])
            pieces = [(si_l, n) for si_l in range(4) for n in range(2)]
            for pi in range(6):
                si_l, n = pieces[pi]
                nc.tensor.matmul(
                    slots[pi], ctp_sb[0][qt][:, 128 * si_l:128 * si_l + 128],
                    wop_sb[0][:, 512 * n:512 * n + 512],
                    start=True, stop=False)
            emit_PV(*last2[1])
            osbs = [smalls.tile([128, 1024], F32, tag="osbt", bufs=4,
                                name="osbt") for _ in range(4)]
            for pi, (si_l, n) in enumerate(pieces):
                op = slots[pi % 6]
                si = 4 * qt + si_l
                if pi >= 6:
                    nc.tensor.matmul(
                        op, ctp_sb[0][qt][:, 128 * si_l:128 * si_l + 128],
                        wop_sb[0][:, 512 * n:512 * n + 512],
                        start=True, stop=False)
                nc.tensor.matmul(
                    op, ctp_sb[1][qt][:, 128 * si_l:128 * si_l + 128],
                    wop_sb[1][:, 512 * n:512 * n + 512],
                    start=False, stop=True)
                eng = nc.scalar.copy if (pi % 2 == 0) else \
                    (lambda d, s: nc.vector.tensor_copy(d, s))
                eng(osbs[si_l][:, 512 * n:512 * n + 512], op)
                if n == 1:
                    nc.sync.dma_start(out[128 * si:128 * si + 128, :],
                                      osbs[si_l][:])

    nc.compile()
    return nc


def _get_nc():
    if MMDT not in _CACHED:
        _CACHED[MMDT] = build_nc()
    return _CACHED[MMDT]


def shard_inputs(x, wq, bq, wk, bk, wv, bv, wo, bo):
    np_mm = _np_mm()
    x = np.asarray(x, dtype=np.float32)
    wq, bq = np.asarray(wq, np.float32), np.asarray(bq, np.float32)
    wk, bk = np.asarray(wk, np.float32), np.asarray(bk, np.float32)
    wv = np.asarray(wv, np.float32)
    wo = np.asarray(wo, np.float32)

    def wlayout(w):  # [D, GD] -> [128, NDK*GD] with (p, m, k, c) order
        return np.ascontiguousarray(
            w.reshape(NDK, 128, 2, 128).transpose(1, 2, 0, 3)
            .reshape(128, NDK * GD)
        ).astype(np_mm)

    def blayout(b):  # [GD] -> [128, 2] with (p, m) order
        return np.ascontiguousarray(b.reshape(2, 128).T).astype(np.float32)

    in_maps = []
    for core in range(8):
        b, g = core // 4, core % 4
        hs = slice(g * GD, (g + 1) * GD)
        in_maps.append({
            "xT": np.ascontiguousarray(x[b].T).astype(np_mm),
            "wq": wlayout(wq[:, hs]),
            "wk": wlayout(wk[:, hs]),
            "wv": wlayout(wv[:, hs]),
            "wo": np.ascontiguousarray(wo[hs, :]).astype(np_mm),
            "bq": blayout(bq[hs]),
            "bk": blayout(bk[hs]),
        })
    return in_maps


def kernel(x, wq, bq, wk, bk, wv, bv, wo, bo, _trace=False, _tracekw=None):
    nc = _get_nc()
    in_maps = shard_inputs(x, wq, bq, wk, bk, wv, bv, wo, bo)
    kw = dict(_tracekw or {})
    res = run_bass_kernel_spmd(nc, in_maps, core_ids=list(range(8)),
                               trace=_trace, **kw)
    kernel._last_result = res
    # constant row: bv @ wo + bo (exact in float64, folded on host)
    bv64 = np.asarray(bv, np.float64)
    wo64 = np.asarray(wo, np.float64)
    bo64 = np.asarray(bo, np.float64)
    const_row = bv64 @ wo64 + bo64
    full = np.empty((2, S, D), np.float32)
    for b in range(2):
        acc = np.zeros((S, D), np.float64)
        for g in range(4):
            acc += res.results[4 * b + g]["out"]
        full[b] = (acc + const_row).astype(np.float32)
    return full


kernel._last_result = None
